# revision 1
# baseline (speedup 1.0000x reference)
"""DiffGRUCell fused kernel for Trainium2 (Bass/Tile), 8-core data-parallel.

Computes, for x = reshape(diffused_x, (B*N, K*F)) and h = h_prev:
    z = sigmoid([x, h] @ Wz + bz)
    r = sigmoid([x, h] @ Wr + br)
    c = tanh([x, r*h] @ Wc + bc)
    h_new = (1 - z) * h + z * c

Sharding: data-parallel over batch. B*N = 20800 tokens are split into 8
contiguous shards of 2600 tokens (8 batches each); gate weights are
replicated. No collectives needed.

Per-core layout strategy:
  - Activations are transposed on-chip (TensorE + identity) into
    feature-major tiles actT[j] = combined.T tile of the 1216-row
    contraction dim, h-part first: j=0..7 -> h[128j:128(j+1)],
    j=8 -> x[0:128], j=9 -> x[128:192] + bias-ones row.
  - Gate biases are folded into the GEMM: weight c-tile j=9 carries the
    bias as an extra row, matched by a constant-1.0 row in actT[9].
  - r is computed feature-major ([h_tile, tokens], weights stationary) so
    rh = sigmoid(r) * h.T is immediately usable as the stationary operand
    of the candidate GEMM.
  - z and c are computed token-major ([tokens, h], activations stationary,
    weights moving) so the final elementwise combine with the
    naturally-loaded h_prev and the output store need no transposes.
"""

import numpy as np

from concourse import bacc
import concourse.mybir as mybir
from concourse.tile import TileContext
from concourse.masks import make_identity
from concourse.bass_utils import run_bass_kernel_spmd

B, N, K, F, H = 64, 325, 3, 64, 1024
XW = K * F            # 192
CONCAT = XW + H       # 1216
NCORES = 8
TPC = (B * N) // NCORES   # 2600 tokens per core
TB = 256                  # token block size
F32 = mybir.dt.float32
MM_DT = mybir.dt.float32r  # matmul compute dtype (float32r: 4x faster PE)


def build(tpc=TPC, tb=TB, mm_dt=MM_DT):
    nc = bacc.Bacc("TRN2")
    x = nc.declare_dram_parameter("x", [tpc, XW], F32, isOutput=False)
    h = nc.declare_dram_parameter("h", [tpc, H], F32, isOutput=False)
    W = {}
    bvec = {}
    for g in "zrc":
        W[g] = nc.declare_dram_parameter(f"W{g}", [CONCAT, H], F32, isOutput=False)
        bvec[g] = nc.declare_dram_parameter(f"b{g}", [H], F32, isOutput=False)
    out = nc.declare_dram_parameter("out", [tpc, H], F32, isOutput=True)

    SIG = mybir.ActivationFunctionType.Sigmoid
    TANH = mybir.ActivationFunctionType.Tanh

    with TileContext(nc) as tc:
        with (
            tc.tile_pool(name="wpool", bufs=1) as wpool,
            tc.tile_pool(name="cpool", bufs=1) as cpool,
            tc.tile_pool(name="xnat", bufs=6) as xpool,
            tc.tile_pool(name="hnat", bufs=5) as hpool,
            tc.tile_pool(name="actT", bufs=20) as apool,
            tc.tile_pool(name="rh", bufs=8) as rhpool,
            tc.tile_pool(name="zsb", bufs=4) as zpool,
            tc.tile_pool(name="csb", bufs=3) as cbpool,
            tc.tile_pool(name="trps", bufs=2, space="PSUM") as trps,
            tc.tile_pool(name="rps", bufs=2, space="PSUM") as rps,
            tc.tile_pool(name="zcps", bufs=4, space="PSUM") as zcps,
        ):
            # Block table: full tb-token blocks, with the sub-128 remainder
            # merged into the final block (a tiny trailing block would pay
            # full LDWEIGHTS cost in its r GEMMs for almost no work).
            nblocks = tpc // tb
            rem = tpc - nblocks * tb
            btbs = [tb] * nblocks
            if rem >= 128 or nblocks == 0:
                btbs.append(rem)
            elif rem:
                btbs[-1] += rem
            blk = []
            t0 = 0
            for btb in btbs:
                blk.append((t0, btb))
                t0 += btb
            max_btb = max(b for _, b in blk)
            order = list(range(len(blk)))

            idt = cpool.tile([128, 128], F32, tag="idt")
            make_identity(nc, idt)
            ones = cpool.tile([1, max_btb], F32, tag="ones")
            nc.vector.memset(ones, 1.0)


            def emit_loads(bidx, chunked=False):
                t0, btb = blk[bidx]
                nsub = (btb + 127) // 128
                xts = []
                hts = []
                for s in range(nsub):
                    r0 = t0 + s * 128
                    ts_ = min(128, t0 + btb - r0)
                    # h before x: the transposes consume the h-part first
                    ht = hpool.tile([128, H], F32, tag="hnat", name=f"hn{bidx}_{s}")
                    if chunked == 2:
                        # very first subtile: 128-col chunks so the first
                        # transpose starts as soon as 64KB lands
                        for k in range(8):
                            nc.sync.dma_start(
                                out=ht[:ts_, 128 * k : 128 * (k + 1)],
                                in_=h[r0 : r0 + ts_, 128 * k : 128 * (k + 1)],
                            )
                        chunked = 1
                    elif chunked:
                        # halve the first transfers so the first transposes
                        # can start ~1.5us earlier at kernel start
                        nc.sync.dma_start(
                            out=ht[:ts_, 0:512], in_=h[r0 : r0 + ts_, 0:512]
                        )
                        nc.sync.dma_start(
                            out=ht[:ts_, 512:H], in_=h[r0 : r0 + ts_, 512:H]
                        )
                    else:
                        nc.sync.dma_start(out=ht[:ts_, :], in_=h[r0 : r0 + ts_, :])
                    xt = xpool.tile([128, XW], F32, tag="xnat", name=f"xn{bidx}_{s}")
                    nc.sync.dma_start(out=xt[:ts_, :], in_=x[r0 : r0 + ts_, :])
                    xts.append((xt, ts_))
                    hts.append((ht, ts_))
                return xts, hts

            def emit_transposes(bidx, xts, hts):
                t0, btb = blk[bidx]
                nsub = (btb + 127) // 128
                a = [
                    apool.tile([128, max_btb], mm_dt, tag="actT", name=f"actT{bidx}_{i}")
                    for i in range(10)
                ]
                nc.scalar.copy(out=a[9][64:65, :btb], in_=ones[:, :btb])
                for s in range(nsub):
                    xt, ts_ = xts[s]
                    ht, _ = hts[s]
                    srcs = [(ht, 128 * k, 128, a[k]) for k in range(8)]
                    srcs += [(xt, 0, 128, a[8]), (xt, 128, 64, a[9])]
                    for src, c0, cw, dst in srcs:
                        pt = trps.tile([128, 128], F32, tag="trps")
                        nc.tensor.transpose(
                            pt[:cw, :ts_], src[:ts_, c0 : c0 + cw], idt[:ts_, :ts_]
                        )
                        nc.vector.tensor_copy(
                            out=dst[0:cw, s * 128 : s * 128 + ts_], in_=pt[:cw, :ts_]
                        )
                return a

            # Prologue: first two blocks' activations load + transpose before
            # any GEMM, so the PE has work while the 15MB of weights stream in.
            def emit_weights(g):
                # h-part tiles first (contraction runs h-first), then x-part
                # and the bias row folded into the final 65-row tile.
                tiles = []
                for k in range(8):
                    t = wpool.tile([128, H], mm_dt, tag=f"w{g}h{k}")
                    nc.sync.dma_start(
                        out=t,
                        in_=W[g][XW + 128 * k : XW + 128 * (k + 1), :].bitcast(mm_dt),
                    )
                    tiles.append((t, 128))
                t = wpool.tile([128, H], mm_dt, tag=f"w{g}0")
                nc.sync.dma_start(out=t, in_=W[g][0:128, :].bitcast(mm_dt))
                tiles.append((t, 128))
                t = wpool.tile([128, H], mm_dt, tag=f"w{g}1")
                nc.sync.dma_start(out=t[0:64, :], in_=W[g][128:192, :].bitcast(mm_dt))
                nc.sync.dma_start(
                    out=t[64:65, :], in_=bvec[g][:].unsqueeze(0).bitcast(mm_dt)
                )
                tiles.append((t, 65))
                return tiles

            # DMA priority order: first two blocks' activations, then the
            # weights in gate-use order (z warms the PE clock gate first).
            state = {}
            wt = {}
            state[order[0]] = emit_loads(order[0], chunked=True)
            if len(order) > 1:
                state[order[1]] = emit_loads(order[1], chunked=True)
            wt["z"] = emit_weights("z")
            wt["r"] = emit_weights("r")
            wt["c"] = emit_weights("c")

            acts = {}
            acts[order[0]] = emit_transposes(order[0], *state[order[0]])
            if len(order) > 1:
                acts[order[1]] = emit_transposes(order[1], *state[order[1]])

            def emit_r(bidx, a):
                btb = blk[bidx][1]
                rh = [
                    rhpool.tile([128, max_btb], mm_dt, tag="rh", name=f"rh{bidx}_{i}")
                    for i in range(8)
                ]
                for k in range(8):
                    pr = rps.tile([128, max_btb], F32, tag="rps")
                    for j, (wtile, kk) in enumerate(wt["r"]):
                        nc.tensor.matmul(
                            pr[:, :btb],
                            lhsT=wtile[:kk, 128 * k : 128 * (k + 1)],
                            rhs=a[j][:kk, :btb],
                            start=(j == 0),
                            stop=(j == 9),
                        )
                    nc.scalar.activation(out=rh[k][:, :btb], in_=pr[:, :btb], func=SIG)
                    nc.vector.tensor_mul(
                        rh[k][:, :btb], rh[k][:, :btb], a[k][:128, :btb]
                    )
                return rh

            def emit_z(bidx, a, xts):
                btb = blk[bidx][1]
                nsub = (btb + 127) // 128
                zts = []
                for s in range(nsub):
                    _, ts_ = xts[s]
                    zt = zpool.tile([128, H], F32, tag="zsb", name=f"z{bidx}_{s}")
                    for hh in range(2):
                        pz = zcps.tile([128, 512], F32, tag="zcps")
                        for j, (wtile, kk) in enumerate(wt["z"]):
                            nc.tensor.matmul(
                                pz[:ts_, :],
                                lhsT=a[j][:kk, s * 128 : s * 128 + ts_],
                                rhs=wtile[:kk, 512 * hh : 512 * (hh + 1)],
                                start=(j == 0),
                                stop=(j == 9),
                            )
                        nc.scalar.activation(
                            out=zt[:ts_, 512 * hh : 512 * (hh + 1)],
                            in_=pz[:ts_, :],
                            func=SIG,
                        )
                    zts.append(zt)
                return zts

            def emit_c(bidx, a, rh, zts, hts, split_tail=False):
                t0, btb = blk[bidx]
                nsub = (btb + 127) // 128
                for s in range(nsub):
                    ht, ts_ = hts[s]
                    ct = cbpool.tile([128, H], F32, tag="csb", name=f"c{bidx}_{s}")
                    r0 = t0 + s * 128
                    # On the final block, combine + store per 512-col half so
                    # the tail chain overlaps the other half's tanh.
                    halfwise = split_tail and s >= nsub - 2
                    for hh in range(2):
                        cs = slice(512 * hh, 512 * (hh + 1))
                        pc = zcps.tile([128, 512], F32, tag="zcps")
                        for j, (wtile, kk) in enumerate(wt["c"]):
                            lhs_src = rh[j] if j < 8 else a[j]
                            nc.tensor.matmul(
                                pc[:ts_, :],
                                lhsT=lhs_src[:kk, s * 128 : s * 128 + ts_],
                                rhs=wtile[:kk, cs],
                                start=(j == 0),
                                stop=(j == 9),
                            )
                        nc.scalar.activation(
                            out=ct[:ts_, cs], in_=pc[:ts_, :], func=TANH
                        )
                        if halfwise:
                            nc.vector.tensor_sub(
                                ct[:ts_, cs], ct[:ts_, cs], ht[:ts_, cs]
                            )
                            nc.vector.tensor_mul(
                                ct[:ts_, cs], ct[:ts_, cs], zts[s][:ts_, cs]
                            )
                            nc.vector.tensor_add(
                                ct[:ts_, cs], ct[:ts_, cs], ht[:ts_, cs]
                            )
                            nc.sync.dma_start(
                                out=out[r0 : r0 + ts_, cs], in_=ct[:ts_, cs]
                            )
                    if not halfwise:
                        # h_new = h + z*(c - h), computed in place in ct
                        nc.vector.tensor_sub(ct[:ts_, :], ct[:ts_, :], ht[:ts_, :])
                        nc.vector.tensor_mul(ct[:ts_, :], ct[:ts_, :], zts[s][:ts_, :])
                        nc.vector.tensor_add(ct[:ts_, :], ct[:ts_, :], ht[:ts_, :])
                        nc.sync.dma_start(out=out[r0 : r0 + ts_, :], in_=ct[:ts_, :])

            if len(order) >= 2:
                # Startup interleave: z1 fills the PE while Wc still streams
                # in, so c0 never exposes a weight-arrival stall.
                b0, b1 = order[0], order[1]
                z0 = emit_z(b0, acts[b0], state[b0][0])
                rh0 = emit_r(b0, acts[b0])
                if len(order) > 2:
                    state[order[2]] = emit_loads(order[2])
                z1 = emit_z(b1, acts[b1], state[b1][0])
                emit_c(b0, acts[b0], rh0, z0, state[b0][1])
                if len(order) > 2:
                    acts[order[2]] = emit_transposes(order[2], *state[order[2]])
                if len(order) > 3:
                    state[order[3]] = emit_loads(order[3])
                rh1 = emit_r(b1, acts[b1])
                emit_c(b1, acts[b1], rh1, z1, state[b1][1])
                acts.pop(b0)
                acts.pop(b1)
                if len(order) > 3:
                    acts[order[3]] = emit_transposes(order[3], *state[order[3]])
                start_i = 2
            else:
                start_i = 0

            for i in range(start_i, len(order)):
                bidx = order[i]
                if i + 2 < len(order):
                    state[order[i + 2]] = emit_loads(order[i + 2])
                a = acts.pop(bidx)
                zts = emit_z(bidx, a, state[bidx][0])
                rh = emit_r(bidx, a)
                emit_c(
                    bidx, a, rh, zts, state[bidx][1],
                    split_tail=(i == len(order) - 1),
                )
                if i + 2 < len(order):
                    acts[order[i + 2]] = emit_transposes(
                        order[i + 2], *state[order[i + 2]]
                    )

    nc.finalize()
    return nc


_NC_CACHE = {}


def _get_nc():
    key = (TPC, TB, str(MM_DT))
    if key not in _NC_CACHE:
        _NC_CACHE[key] = build()
    return _NC_CACHE[key]


def _make_in_maps(diffused_x, h_prev, Wz, bz, Wr, br, Wc, bc, tpc=TPC):
    x = np.ascontiguousarray(
        np.asarray(diffused_x, dtype=np.float32).reshape(B * N, XW)
    )
    hp = np.ascontiguousarray(np.asarray(h_prev, dtype=np.float32).reshape(B * N, H))
    shared = {
        "Wz": Wz, "bz": bz, "Wr": Wr, "br": br, "Wc": Wc, "bc": bc,
    }
    shared = {
        k: np.ascontiguousarray(np.asarray(v, dtype=np.float32))
        for k, v in shared.items()
    }
    in_maps = []
    for c in range(NCORES):
        sl = slice(c * tpc, (c + 1) * tpc)
        m = {"x": x[sl], "h": hp[sl]}
        m.update(shared)
        in_maps.append(m)
    return in_maps


def kernel(diffused_x, h_prev, Wz, bz, Wr, br, Wc, bc):
    nc = _get_nc()
    in_maps = _make_in_maps(diffused_x, h_prev, Wz, bz, Wr, br, Wc, bc)
    res = run_bass_kernel_spmd(nc, in_maps, list(range(NCORES)))
    outs = [res.results[c]["out"] for c in range(NCORES)]
    return np.concatenate(outs, axis=0).reshape(B, N, H)


def kernel_traced(diffused_x, h_prev, Wz, bz, Wr, br, Wc, bc):
    """Like kernel() but with NTFF profiling; returns (out, BassKernelResults)."""
    nc = _get_nc()
    in_maps = _make_in_maps(diffused_x, h_prev, Wz, bz, Wr, br, Wc, bc)
    res = run_bass_kernel_spmd(nc, in_maps, list(range(NCORES)), trace=True)
    outs = [res.results[c]["out"] for c in range(NCORES)]
    return np.concatenate(outs, axis=0).reshape(B, N, H), res



# revision 3
# speedup vs baseline: 1.2309x; 1.2309x over previous
"""DiffGRUCell fused kernel for Trainium2 (Bass/Tile), 8-core data-parallel.

Computes, for x = reshape(diffused_x, (B*N, K*F)) and h = h_prev:
    z = sigmoid([x, h] @ Wz + bz)
    r = sigmoid([x, h] @ Wr + br)
    c = tanh([x, r*h] @ Wc + bc)
    h_new = (1 - z) * h + z * c

Sharding: data-parallel over batch. B*N = 20800 tokens are split into 8
contiguous shards of 2600 tokens; gate weights are replicated. No
collectives needed.

Everything on the GEMM path runs in bf16 (tolerance is 2e-2; bf16 lands
~5e-3): weights are packed host-side as [h-part, x-part, bias-row] rows
and cast to bf16 (halves the 15MB weight stream and enables FWL weight
loads), activations and the output are bf16 in DRAM, and the final
combine runs bf16 on the DVE at 2x rate.

Per-core layout strategy:
  - Activations are transposed on-chip (TensorE + bf16 identity) into
    feature-major tiles actT[j] = combined.T of the 1217-row contraction
    (h-part j=0..7, x-part j=8, j=9 = x[128:192] + bias-ones row). The
    4 per-subtile transposes of one j share a single PSUM bank and drain
    with one wide DVE copy.
  - Gate biases ride the GEMM via the packed weight row 1216 against the
    constant-1.0 row in actT[9].
  - r is computed feature-major (weights stationary) so rh = sigmoid(r)
    * h.T feeds the candidate GEMM as its stationary operand.
  - z and c are computed token-major (activations stationary, weights
    moving) so the combine uses the naturally-loaded h tiles directly.
  - Startup: weight DMA is issued first (z, r, c order) and the c-GEMM
    lags one block so the PE is dense while Wc streams in. The first
    block (296 tokens) absorbs the 40-token tail during the cold-clock
    window; the last block is small (256) to shorten the drain tail.
"""

import numpy as np
import ml_dtypes

from concourse import bacc
import concourse.mybir as mybir
from concourse.tile import TileContext
from concourse.masks import make_identity
from concourse.bass_utils import run_bass_kernel_spmd

B, N, K, F, H = 64, 325, 3, 64, 1024
XW = K * F             # 192
CONCAT = XW + H        # 1216
CROWS = CONCAT + 1     # 1217 packed weight rows (h, x, bias)
NCORES = 8
TPC = (B * N) // NCORES    # 2600 tokens per core
F32 = mybir.dt.float32
BF16 = mybir.dt.bfloat16

# Block table: first block absorbs the sub-128 remainder during the cold
# window; last block small so the store tail is short.
BLOCKS = [296, 512, 512, 512, 512, 256]
assert sum(BLOCKS) == TPC


def build():
    nc = bacc.Bacc("TRN2")
    x = nc.declare_dram_parameter("x", [TPC, XW], BF16, isOutput=False)
    h = nc.declare_dram_parameter("h", [TPC, H], BF16, isOutput=False)
    W = {}
    for g in "zrc":
        W[g] = nc.declare_dram_parameter(f"W{g}", [CROWS, H], BF16, isOutput=False)
    out = nc.declare_dram_parameter("out", [TPC, H], BF16, isOutput=True)

    SIG = mybir.ActivationFunctionType.Sigmoid
    TANH = mybir.ActivationFunctionType.Tanh

    blk = []
    t0 = 0
    for btb in BLOCKS:
        blk.append((t0, btb))
        t0 += btb
    nb = len(blk)

    with TileContext(nc) as tc:
        with (
            tc.tile_pool(name="wpool", bufs=1) as wpool,
            tc.tile_pool(name="cpool", bufs=1) as cpool,
            tc.tile_pool(name="xnat", bufs=12) as xpool,
            tc.tile_pool(name="hnat", bufs=12) as hpool,
            tc.tile_pool(name="actT", bufs=30) as apool,
            tc.tile_pool(name="rh", bufs=20) as rhpool,
            tc.tile_pool(name="zsb", bufs=10) as zpool,
            tc.tile_pool(name="csb", bufs=6) as cbpool,
            tc.tile_pool(name="trps", bufs=2, space="PSUM") as trps,
            tc.tile_pool(name="rps", bufs=2, space="PSUM") as rps,
            tc.tile_pool(name="zcps", bufs=4, space="PSUM") as zcps,
        ):
            idt = cpool.tile([128, 128], BF16, tag="idt")
            make_identity(nc, idt)
            ones = cpool.tile([1, 512], BF16, tag="ones")
            nc.vector.memset(ones, 1.0)

            def emit_weights(g):
                tiles = []
                for j in range(10):
                    kk = 128 if j < 9 else 65
                    t = wpool.tile([128, H], BF16, tag=f"w{g}{j}", name=f"w{g}{j}")
                    nc.sync.dma_start(
                        out=t[:kk, :], in_=W[g][128 * j : 128 * j + kk, :]
                    )
                    tiles.append((t, kk))
                return tiles

            def emit_loads(bidx, chunked=False):
                t0, btb = blk[bidx]
                nsub = (btb + 127) // 128
                xts = []
                hts = []
                for s in range(nsub):
                    r0 = t0 + s * 128
                    ts_ = min(128, t0 + btb - r0)
                    ht = hpool.tile([128, H], BF16, tag="hnat", name=f"hn{bidx}_{s}")
                    if chunked:
                        # 128-col chunks aligned with the j-tiles so the
                        # first transposes start as soon as 32KB lands
                        for k in range(8):
                            nc.sync.dma_start(
                                out=ht[:ts_, 128 * k : 128 * (k + 1)],
                                in_=h[r0 : r0 + ts_, 128 * k : 128 * (k + 1)],
                            )
                    else:
                        nc.sync.dma_start(out=ht[:ts_, :], in_=h[r0 : r0 + ts_, :])
                    xt = xpool.tile([128, XW], BF16, tag="xnat", name=f"xn{bidx}_{s}")
                    nc.sync.dma_start(out=xt[:ts_, :], in_=x[r0 : r0 + ts_, :])
                    xts.append((xt, ts_))
                    hts.append((ht, ts_))
                return xts, hts

            def emit_transposes(bidx, xts, hts):
                t0, btb = blk[bidx]
                nsub = (btb + 127) // 128
                a = [
                    apool.tile([128, 512], BF16, tag="actT", name=f"actT{bidx}_{j}")
                    for j in range(10)
                ]
                nc.scalar.copy(out=a[9][64:65, :btb], in_=ones[:, :btb])
                for j in range(10):
                    if j < 8:
                        c0, cw = 128 * j, 128
                        srcs = hts
                    elif j == 8:
                        c0, cw = 0, 128
                        srcs = xts
                    else:
                        c0, cw = 128, 64
                        srcs = xts
                    pt = trps.tile([128, 512], BF16, tag="trps")
                    for s in range(nsub):
                        src, ts_ = srcs[s]
                        nc.tensor.transpose(
                            pt[:cw, s * 128 : s * 128 + ts_],
                            src[:ts_, c0 : c0 + cw],
                            idt[:ts_, :ts_],
                        )
                    nc.vector.tensor_copy(out=a[j][:cw, :btb], in_=pt[:cw, :btb])
                return a

            def emit_r(bidx, a, wt):
                btb = blk[bidx][1]
                rh = [
                    rhpool.tile([128, 512], BF16, tag="rh", name=f"rh{bidx}_{k}")
                    for k in range(8)
                ]
                for k in range(8):
                    pr = rps.tile([128, 512], F32, tag="rps")
                    for j, (wtile, kk) in enumerate(wt):
                        nc.tensor.matmul(
                            pr[:, :btb],
                            lhsT=wtile[:kk, 128 * k : 128 * (k + 1)],
                            rhs=a[j][:kk, :btb],
                            start=(j == 0),
                            stop=(j == 9),
                        )
                    nc.scalar.activation(out=rh[k][:, :btb], in_=pr[:, :btb], func=SIG)
                    nc.vector.tensor_mul(
                        rh[k][:, :btb], rh[k][:, :btb], a[k][:128, :btb]
                    )
                return rh

            def emit_z(bidx, a, wt, nsubs):
                btb = blk[bidx][1]
                nsub = (btb + 127) // 128
                zts = []
                for s in range(nsub):
                    ts_ = nsubs[s]
                    zt = zpool.tile([128, H], BF16, tag="zsb", name=f"z{bidx}_{s}")
                    for hh in range(2):
                        pz = zcps.tile([128, 512], F32, tag="zcps")
                        for j, (wtile, kk) in enumerate(wt):
                            nc.tensor.matmul(
                                pz[:ts_, :],
                                lhsT=a[j][:kk, s * 128 : s * 128 + ts_],
                                rhs=wtile[:kk, 512 * hh : 512 * (hh + 1)],
                                start=(j == 0),
                                stop=(j == 9),
                            )
                        nc.scalar.activation(
                            out=zt[:ts_, 512 * hh : 512 * (hh + 1)],
                            in_=pz[:ts_, :],
                            func=SIG,
                        )
                    zts.append(zt)
                return zts

            def emit_c(bidx, a, rh, zts, hts, wt):
                t0, btb = blk[bidx]
                nsub = (btb + 127) // 128
                for s in range(nsub):
                    ht, ts_ = hts[s]
                    ct = cbpool.tile([128, H], BF16, tag="csb", name=f"c{bidx}_{s}")
                    r0 = t0 + s * 128
                    for hh in range(2):
                        cs = slice(512 * hh, 512 * (hh + 1))
                        pc = zcps.tile([128, 512], F32, tag="zcps")
                        for j, (wtile, kk) in enumerate(wt):
                            lhs_src = rh[j] if j < 8 else a[j]
                            nc.tensor.matmul(
                                pc[:ts_, :],
                                lhsT=lhs_src[:kk, s * 128 : s * 128 + ts_],
                                rhs=wtile[:kk, cs],
                                start=(j == 0),
                                stop=(j == 9),
                            )
                        nc.scalar.activation(
                            out=ct[:ts_, cs], in_=pc[:ts_, :], func=TANH
                        )
                    # h_new = h + z*(c - h), in place in ct (all bf16)
                    nc.vector.tensor_sub(ct[:ts_, :], ct[:ts_, :], ht[:ts_, :])
                    nc.vector.tensor_mul(ct[:ts_, :], ct[:ts_, :], zts[s][:ts_, :])
                    nc.vector.tensor_add(ct[:ts_, :], ct[:ts_, :], ht[:ts_, :])
                    nc.sync.dma_start(out=out[r0 : r0 + ts_, :], in_=ct[:ts_, :])

            def subs(bidx):
                t0, btb = blk[bidx]
                nsub = (btb + 127) // 128
                return [min(128, btb - s * 128) for s in range(nsub)]

            # ---- startup: activations for b0/b1 first (first transposes),
            # then weights in gate-use order; c lags one block throughout.
            state = {}
            state[0] = emit_loads(0, chunked=True)
            state[1] = emit_loads(1, chunked=True)
            wt = {g: emit_weights(g) for g in "zrc"}
            acts = {}
            acts[0] = emit_transposes(0, *state[0])
            acts[1] = emit_transposes(1, *state[1])

            zs = {}
            rhs = {}
            zs[0] = emit_z(0, acts[0], wt["z"], subs(0))
            rhs[0] = emit_r(0, acts[0], wt["r"])
            state[2] = emit_loads(2)
            zs[1] = emit_z(1, acts[1], wt["z"], subs(1))
            rhs[1] = emit_r(1, acts[1], wt["r"])
            emit_c(0, acts[0], rhs[0], zs[0], state[0][1], wt["c"])
            acts[2] = emit_transposes(2, *state[2])

            for i in range(2, nb):
                if i + 1 < nb:
                    state[i + 1] = emit_loads(i + 1)
                zs[i] = emit_z(i, acts[i], wt["z"], subs(i))
                rhs[i] = emit_r(i, acts[i], wt["r"])
                emit_c(
                    i - 1, acts[i - 1], rhs[i - 1], zs[i - 1], state[i - 1][1],
                    wt["c"],
                )
                if i + 1 < nb:
                    acts[i + 1] = emit_transposes(i + 1, *state[i + 1])
            emit_c(
                nb - 1, acts[nb - 1], rhs[nb - 1], zs[nb - 1], state[nb - 1][1],
                wt["c"],
            )

    nc.finalize()
    return nc


_NC_CACHE = {}


def _get_nc():
    if "nc" not in _NC_CACHE:
        _NC_CACHE["nc"] = build()
    return _NC_CACHE["nc"]


def _pack_weights(Wg, bg):
    """Rows reordered [h-part, x-part, bias] and cast to bf16."""
    Wg = np.asarray(Wg, dtype=np.float32)
    bg = np.asarray(bg, dtype=np.float32)
    Wp = np.empty((CROWS, H), dtype=np.float32)
    Wp[0:H] = Wg[XW:]
    Wp[H : H + XW] = Wg[:XW]
    Wp[H + XW] = bg
    return np.ascontiguousarray(Wp.astype(ml_dtypes.bfloat16))


def _make_in_maps(diffused_x, h_prev, Wz, bz, Wr, br, Wc, bc):
    x = np.ascontiguousarray(
        np.asarray(diffused_x, dtype=np.float32)
        .reshape(B * N, XW)
        .astype(ml_dtypes.bfloat16)
    )
    hp = np.ascontiguousarray(
        np.asarray(h_prev, dtype=np.float32)
        .reshape(B * N, H)
        .astype(ml_dtypes.bfloat16)
    )
    shared = {
        "Wz": _pack_weights(Wz, bz),
        "Wr": _pack_weights(Wr, br),
        "Wc": _pack_weights(Wc, bc),
    }
    in_maps = []
    for c in range(NCORES):
        sl = slice(c * TPC, (c + 1) * TPC)
        m = {"x": x[sl], "h": hp[sl]}
        m.update(shared)
        in_maps.append(m)
    return in_maps


def kernel(diffused_x, h_prev, Wz, bz, Wr, br, Wc, bc):
    nc = _get_nc()
    in_maps = _make_in_maps(diffused_x, h_prev, Wz, bz, Wr, br, Wc, bc)
    res = run_bass_kernel_spmd(nc, in_maps, list(range(NCORES)))
    outs = [res.results[c]["out"] for c in range(NCORES)]
    return np.concatenate(outs, axis=0).astype(np.float32).reshape(B, N, H)


def kernel_traced(diffused_x, h_prev, Wz, bz, Wr, br, Wc, bc):
    """Like kernel() but with NTFF profiling; returns (out, BassKernelResults)."""
    nc = _get_nc()
    in_maps = _make_in_maps(diffused_x, h_prev, Wz, bz, Wr, br, Wc, bc)
    res = run_bass_kernel_spmd(nc, in_maps, list(range(NCORES)), trace=True)
    outs = [res.results[c]["out"] for c in range(NCORES)]
    return (
        np.concatenate(outs, axis=0).astype(np.float32).reshape(B, N, H),
        res,
    )


# revision 8
# speedup vs baseline: 1.3391x; 1.0880x over previous
"""DiffGRUCell fused kernel for Trainium2 (Bass/Tile), 8-core data-parallel.

Computes, for x = reshape(diffused_x, (B*N, K*F)) and h = h_prev:
    z = sigmoid([x, h] @ Wz + bz)
    r = sigmoid([x, h] @ Wr + br)
    c = tanh([x, r*h] @ Wc + bc)
    h_new = (1 - z) * h + z * c

Sharding: data-parallel over batch. B*N = 20800 tokens are split into 8
contiguous shards of 2600 tokens; gate weights are replicated. No
collectives needed.

Everything on the GEMM path runs in bf16 (tolerance is 2e-2; bf16 lands
~5e-3): weights are packed host-side as [h-part, x-part, bias-row] rows
and cast to bf16 (halves the 15MB weight stream and enables FWL weight
loads), activations and the output are bf16 in DRAM, and the final
combine runs bf16 on the DVE at 2x rate.

Per-core layout strategy:
  - Activations are transposed on-chip (TensorE + bf16 identity) into
    feature-major tiles actT[j] = combined.T of the 1217-row contraction
    (h-part j=0..7, x-part j=8, j=9 = x[128:192] + bias-ones row). The
    4 per-subtile transposes of one j share a single PSUM bank and drain
    with one wide DVE copy.
  - Gate biases ride the GEMM via the packed weight row 1216 against the
    constant-1.0 row in actT[9].
  - r is computed feature-major (weights stationary) so rh = sigmoid(r)
    * h.T feeds the candidate GEMM as its stationary operand.
  - z and c are computed token-major (activations stationary, weights
    moving) so the combine uses the naturally-loaded h tiles directly.
  - Startup: weight DMA is issued first (z, r, c order) and the c-GEMM
    lags one block so the PE is dense while Wc streams in. The first
    block (296 tokens) absorbs the 40-token tail during the cold-clock
    window; the last block is small (256) to shorten the drain tail.
"""

import numpy as np
import ml_dtypes

from concourse import bacc
import concourse.mybir as mybir
from concourse.tile import TileContext
from concourse.masks import make_identity
from concourse.bass_utils import run_bass_kernel_spmd

B, N, K, F, H = 64, 325, 3, 64, 1024
XW = K * F             # 192
CONCAT = XW + H        # 1216
CROWS = CONCAT + 1     # 1217 packed weight rows (h, x, bias)
NCORES = 8
TPC = (B * N) // NCORES    # 2600 tokens per core
F32 = mybir.dt.float32
BF16 = mybir.dt.bfloat16

# Block table: first block absorbs the sub-128 remainder during the cold
# window; last block small so the store tail is short.
BLOCKS = [296, 512, 512, 512, 512, 256]
assert sum(BLOCKS) == TPC


def build():
    nc = bacc.Bacc("TRN2")
    x = nc.declare_dram_parameter("x", [TPC, XW], BF16, isOutput=False)
    h = nc.declare_dram_parameter("h", [TPC, H], BF16, isOutput=False)
    W = {}
    for g in "zrc":
        W[g] = nc.declare_dram_parameter(f"W{g}", [CROWS, H], BF16, isOutput=False)
    out = nc.declare_dram_parameter("out", [TPC, H], BF16, isOutput=True)

    SIG = mybir.ActivationFunctionType.Sigmoid
    TANH = mybir.ActivationFunctionType.Tanh

    blk = []
    t0 = 0
    for btb in BLOCKS:
        blk.append((t0, btb))
        t0 += btb
    nb = len(blk)

    with TileContext(nc) as tc:
        with (
            tc.tile_pool(name="wpool", bufs=1) as wpool,
            tc.tile_pool(name="cpool", bufs=1) as cpool,
            tc.tile_pool(name="xnat", bufs=16) as xpool,
            tc.tile_pool(name="hnat", bufs=16) as hpool,
            tc.tile_pool(name="actT", bufs=40) as apool,
            tc.tile_pool(name="rh", bufs=20) as rhpool,
            tc.tile_pool(name="zsb", bufs=10) as zpool,
            tc.tile_pool(name="csb", bufs=6) as cbpool,
            tc.tile_pool(name="trps", bufs=2, space="PSUM") as trps,
            tc.tile_pool(name="rps", bufs=2, space="PSUM") as rps,
            tc.tile_pool(name="zcps", bufs=4, space="PSUM") as zcps,
        ):
            idt = cpool.tile([128, 128], BF16, tag="idt")
            make_identity(nc, idt)
            ones = cpool.tile([1, 512], BF16, tag="ones")
            nc.vector.memset(ones, 1.0)

            def emit_weights(g):
                tiles = []
                for j in range(10):
                    kk = 128 if j < 9 else 65
                    t = wpool.tile([128, H], BF16, tag=f"w{g}{j}", name=f"w{g}{j}")
                    nc.sync.dma_start(
                        out=t[:kk, :], in_=W[g][128 * j : 128 * j + kk, :]
                    )
                    tiles.append((t, kk))
                return tiles

            def emit_loads(bidx):
                # gpsimd queue: stays off the weight stream's (sync) queue.
                # Full-row transfers only — 128-col chunks would drop the
                # DMA to ~50GB/s (256B partition lines).
                t0, btb = blk[bidx]
                nsub = (btb + 127) // 128
                xts = []
                hts = []
                for s in range(nsub):
                    r0 = t0 + s * 128
                    ts_ = min(128, t0 + btb - r0)
                    ht = hpool.tile([128, H], BF16, tag="hnat", name=f"hn{bidx}_{s}")
                    nc.gpsimd.dma_start(out=ht[:ts_, :], in_=h[r0 : r0 + ts_, :])
                    xt = xpool.tile([128, XW], BF16, tag="xnat", name=f"xn{bidx}_{s}")
                    nc.gpsimd.dma_start(out=xt[:ts_, :], in_=x[r0 : r0 + ts_, :])
                    xts.append((xt, ts_))
                    hts.append((ht, ts_))
                return xts, hts

            def emit_transposes(bidx, xts, hts):
                t0, btb = blk[bidx]
                nsub = (btb + 127) // 128
                a = [
                    apool.tile([128, 512], BF16, tag="actT", name=f"actT{bidx}_{j}")
                    for j in range(10)
                ]
                nc.scalar.copy(out=a[9][64:65, :btb], in_=ones[:, :btb])
                for j in range(10):
                    if j < 8:
                        c0, cw = 128 * j, 128
                        srcs = hts
                    elif j == 8:
                        c0, cw = 0, 128
                        srcs = xts
                    else:
                        c0, cw = 128, 64
                        srcs = xts
                    pt = trps.tile([128, 512], BF16, tag="trps")
                    for s in range(nsub):
                        src, ts_ = srcs[s]
                        nc.tensor.transpose(
                            pt[:cw, s * 128 : s * 128 + ts_],
                            src[:ts_, c0 : c0 + cw],
                            idt[:ts_, :ts_],
                        )
                    nc.vector.tensor_copy(out=a[j][:cw, :btb], in_=pt[:cw, :btb])
                return a

            def emit_r(bidx, a, wt):
                btb = blk[bidx][1]
                rh = [
                    rhpool.tile([128, 512], BF16, tag="rh", name=f"rh{bidx}_{k}")
                    for k in range(8)
                ]
                for k in range(8):
                    pr = rps.tile([128, 512], F32, tag="rps")
                    for j, (wtile, kk) in enumerate(wt):
                        nc.tensor.matmul(
                            pr[:, :btb],
                            lhsT=wtile[:kk, 128 * k : 128 * (k + 1)],
                            rhs=a[j][:kk, :btb],
                            start=(j == 0),
                            stop=(j == 9),
                        )
                    nc.scalar.activation(out=rh[k][:, :btb], in_=pr[:, :btb], func=SIG)
                    nc.vector.tensor_mul(
                        rh[k][:, :btb], rh[k][:, :btb], a[k][:128, :btb]
                    )
                return rh

            def emit_z(bidx, a, wt, nsubs):
                btb = blk[bidx][1]
                nsub = (btb + 127) // 128
                zts = []
                for s in range(nsub):
                    ts_ = nsubs[s]
                    zt = zpool.tile([128, H], BF16, tag="zsb", name=f"z{bidx}_{s}")
                    for hh in range(2):
                        pz = zcps.tile([128, 512], F32, tag="zcps")
                        for j, (wtile, kk) in enumerate(wt):
                            nc.tensor.matmul(
                                pz[:ts_, :],
                                lhsT=a[j][:kk, s * 128 : s * 128 + ts_],
                                rhs=wtile[:kk, 512 * hh : 512 * (hh + 1)],
                                start=(j == 0),
                                stop=(j == 9),
                            )
                        nc.scalar.activation(
                            out=zt[:ts_, 512 * hh : 512 * (hh + 1)],
                            in_=pz[:ts_, :],
                            func=SIG,
                        )
                    zts.append(zt)
                return zts

            def emit_c(bidx, a, rh, zts, hts, wt, split_tail=False):
                t0, btb = blk[bidx]
                nsub = (btb + 127) // 128
                for s in range(nsub):
                    ht, ts_ = hts[s]
                    ct = cbpool.tile([128, H], BF16, tag="csb", name=f"c{bidx}_{s}")
                    r0 = t0 + s * 128
                    halfwise = split_tail and s == nsub - 1
                    for hh in range(2):
                        cs = slice(512 * hh, 512 * (hh + 1))
                        pc = zcps.tile([128, 512], F32, tag="zcps")
                        for j, (wtile, kk) in enumerate(wt):
                            lhs_src = rh[j] if j < 8 else a[j]
                            nc.tensor.matmul(
                                pc[:ts_, :],
                                lhsT=lhs_src[:kk, s * 128 : s * 128 + ts_],
                                rhs=wtile[:kk, cs],
                                start=(j == 0),
                                stop=(j == 9),
                            )
                        nc.scalar.activation(
                            out=ct[:ts_, cs], in_=pc[:ts_, :], func=TANH
                        )
                        if halfwise:
                            # tail: combine+store per half so the last store
                            # overlaps the other half's tanh
                            nc.vector.tensor_sub(
                                ct[:ts_, cs], ct[:ts_, cs], ht[:ts_, cs]
                            )
                            nc.vector.tensor_mul(
                                ct[:ts_, cs], ct[:ts_, cs], zts[s][:ts_, cs]
                            )
                            nc.vector.tensor_add(
                                ct[:ts_, cs], ct[:ts_, cs], ht[:ts_, cs]
                            )
                            nc.gpsimd.dma_start(
                                out=out[r0 : r0 + ts_, cs], in_=ct[:ts_, cs]
                            )
                    if not halfwise:
                        # h_new = h + z*(c - h), in place in ct (all bf16)
                        nc.vector.tensor_sub(ct[:ts_, :], ct[:ts_, :], ht[:ts_, :])
                        nc.vector.tensor_mul(ct[:ts_, :], ct[:ts_, :], zts[s][:ts_, :])
                        nc.vector.tensor_add(ct[:ts_, :], ct[:ts_, :], ht[:ts_, :])
                        nc.gpsimd.dma_start(out=out[r0 : r0 + ts_, :], in_=ct[:ts_, :])

            def subs(bidx):
                t0, btb = blk[bidx]
                nsub = (btb + 127) // 128
                return [min(128, btb - s * 128) for s in range(nsub)]

            # ---- startup: activations stream on the gpsimd queue while
            # weights stream on the sync queue; transposes (which need no
            # weights) are interleaved between the first gate GEMMs so the
            # PE has filler work while each gate's weights land; c lags one
            # block throughout.
            state = {}
            state[0] = emit_loads(0)
            state[1] = emit_loads(1)
            wt = {g: emit_weights(g) for g in "zrc"}
            state[2] = emit_loads(2)
            state[3] = emit_loads(3)
            acts = {}
            acts[0] = emit_transposes(0, *state[0])
            acts[1] = emit_transposes(1, *state[1])

            zs = {}
            rhs = {}
            zs[0] = emit_z(0, acts[0], wt["z"], subs(0))
            acts[2] = emit_transposes(2, *state[2])
            rhs[0] = emit_r(0, acts[0], wt["r"])
            acts[3] = emit_transposes(3, *state[3])
            zs[1] = emit_z(1, acts[1], wt["z"], subs(1))
            rhs[1] = emit_r(1, acts[1], wt["r"])
            emit_c(0, acts[0], rhs[0], zs[0], state[0][1], wt["c"])

            for i in range(2, nb):
                if i + 2 < nb:
                    state[i + 2] = emit_loads(i + 2)
                zs[i] = emit_z(i, acts[i], wt["z"], subs(i))
                rhs[i] = emit_r(i, acts[i], wt["r"])
                emit_c(
                    i - 1, acts[i - 1], rhs[i - 1], zs[i - 1], state[i - 1][1],
                    wt["c"],
                )
                if i + 2 < nb:
                    acts[i + 2] = emit_transposes(i + 2, *state[i + 2])
            emit_c(
                nb - 1, acts[nb - 1], rhs[nb - 1], zs[nb - 1], state[nb - 1][1],
                wt["c"], split_tail=True,
            )

    nc.finalize()
    return nc


_NC_CACHE = {}


def _get_nc():
    if "nc" not in _NC_CACHE:
        _NC_CACHE["nc"] = build()
    return _NC_CACHE["nc"]


def _pack_weights(Wg, bg):
    """Rows reordered [h-part, x-part, bias] and cast to bf16."""
    Wg = np.asarray(Wg, dtype=np.float32)
    bg = np.asarray(bg, dtype=np.float32)
    Wp = np.empty((CROWS, H), dtype=np.float32)
    Wp[0:H] = Wg[XW:]
    Wp[H : H + XW] = Wg[:XW]
    Wp[H + XW] = bg
    return np.ascontiguousarray(Wp.astype(ml_dtypes.bfloat16))


def _make_in_maps(diffused_x, h_prev, Wz, bz, Wr, br, Wc, bc):
    x = np.ascontiguousarray(
        np.asarray(diffused_x, dtype=np.float32)
        .reshape(B * N, XW)
        .astype(ml_dtypes.bfloat16)
    )
    hp = np.ascontiguousarray(
        np.asarray(h_prev, dtype=np.float32)
        .reshape(B * N, H)
        .astype(ml_dtypes.bfloat16)
    )
    shared = {
        "Wz": _pack_weights(Wz, bz),
        "Wr": _pack_weights(Wr, br),
        "Wc": _pack_weights(Wc, bc),
    }
    in_maps = []
    for c in range(NCORES):
        sl = slice(c * TPC, (c + 1) * TPC)
        m = {"x": x[sl], "h": hp[sl]}
        m.update(shared)
        in_maps.append(m)
    return in_maps


def kernel(diffused_x, h_prev, Wz, bz, Wr, br, Wc, bc):
    nc = _get_nc()
    in_maps = _make_in_maps(diffused_x, h_prev, Wz, bz, Wr, br, Wc, bc)
    res = run_bass_kernel_spmd(nc, in_maps, list(range(NCORES)))
    outs = [res.results[c]["out"] for c in range(NCORES)]
    return np.concatenate(outs, axis=0).astype(np.float32).reshape(B, N, H)


def kernel_traced(diffused_x, h_prev, Wz, bz, Wr, br, Wc, bc):
    """Like kernel() but with NTFF profiling; returns (out, BassKernelResults)."""
    nc = _get_nc()
    in_maps = _make_in_maps(diffused_x, h_prev, Wz, bz, Wr, br, Wc, bc)
    res = run_bass_kernel_spmd(nc, in_maps, list(range(NCORES)), trace=True)
    outs = [res.results[c]["out"] for c in range(NCORES)]
    return (
        np.concatenate(outs, axis=0).astype(np.float32).reshape(B, N, H),
        res,
    )


# revision 11
# speedup vs baseline: 1.3620x; 1.0171x over previous
"""DiffGRUCell fused kernel for Trainium2 (Bass/Tile), 8-core data-parallel.

Computes, for x = reshape(diffused_x, (B*N, K*F)) and h = h_prev:
    z = sigmoid([x, h] @ Wz + bz)
    r = sigmoid([x, h] @ Wr + br)
    c = tanh([x, r*h] @ Wc + bc)
    h_new = (1 - z) * h + z * c

Sharding: data-parallel over batch. B*N = 20800 tokens are split into 8
contiguous shards of 2600 tokens; gate weights are replicated. No
collectives needed.

Everything on the GEMM path runs in bf16 (tolerance is 2e-2; bf16 lands
~5e-3): weights are packed host-side as [h-part, x-part, bias-row] rows
and cast to bf16 (halves the 15MB weight stream and enables FWL weight
loads), activations and the output are bf16 in DRAM, and the final
combine runs bf16 on the DVE at 2x rate.

Per-core layout strategy:
  - Activations are transposed on-chip (TensorE + bf16 identity) into
    feature-major tiles actT[j] = combined.T of the 1217-row contraction
    (h-part j=0..7, x-part j=8, j=9 = x[128:192] + bias-ones row). The
    4 per-subtile transposes of one j share a single PSUM bank and drain
    with one wide DVE copy.
  - Gate biases ride the GEMM via the packed weight row 1216 against the
    constant-1.0 row in actT[9].
  - r is computed feature-major (weights stationary) so rh = sigmoid(r)
    * h.T feeds the candidate GEMM as its stationary operand.
  - z and c are computed token-major (activations stationary, weights
    moving) so the combine uses the naturally-loaded h tiles directly.
  - Startup: weight DMA is issued first (z, r, c order) and the c-GEMM
    lags one block so the PE is dense while Wc streams in. The first
    block (296 tokens) absorbs the 40-token tail during the cold-clock
    window; the last block is small (256) to shorten the drain tail.
"""

import numpy as np
import ml_dtypes

from concourse import bacc
import concourse.mybir as mybir
from concourse.tile import TileContext
from concourse.masks import make_identity
from concourse.bass_utils import run_bass_kernel_spmd

B, N, K, F, H = 64, 325, 3, 64, 1024
XW = K * F             # 192
CONCAT = XW + H        # 1216
CROWS = CONCAT + 1     # 1217 packed weight rows (h, x, bias)
NCORES = 8
TPC = (B * N) // NCORES    # 2600 tokens per core
F32 = mybir.dt.float32
BF16 = mybir.dt.bfloat16

# Block table: tiny first block (fast first transposes + a z-GEMM that
# paces with the Wz stream), the sub-128 remainder absorbed while the
# weight stream still gates the PE, tiny last block for a short tail.
BLOCKS = [128, 296, 512, 512, 512, 512, 128]
assert sum(BLOCKS) == TPC


def build():
    nc = bacc.Bacc("TRN2")
    x = nc.declare_dram_parameter("x", [TPC, XW], BF16, isOutput=False)
    h = nc.declare_dram_parameter("h", [TPC, H], BF16, isOutput=False)
    W = {}
    for g in "zrc":
        W[g] = nc.declare_dram_parameter(f"W{g}", [CROWS, H], BF16, isOutput=False)
    out = nc.declare_dram_parameter("out", [TPC, H], BF16, isOutput=True)

    SIG = mybir.ActivationFunctionType.Sigmoid
    TANH = mybir.ActivationFunctionType.Tanh

    blk = []
    t0 = 0
    for btb in BLOCKS:
        blk.append((t0, btb))
        t0 += btb
    nb = len(blk)

    with TileContext(nc) as tc:
        with (
            tc.tile_pool(name="wpool", bufs=1) as wpool,
            tc.tile_pool(name="cpool", bufs=1) as cpool,
            tc.tile_pool(name="xnat", bufs=16) as xpool,
            tc.tile_pool(name="hnat", bufs=16) as hpool,
            tc.tile_pool(name="actT", bufs=40) as apool,
            tc.tile_pool(name="rh", bufs=20) as rhpool,
            tc.tile_pool(name="zsb", bufs=10) as zpool,
            tc.tile_pool(name="csb", bufs=6) as cbpool,
            tc.tile_pool(name="trps", bufs=2, space="PSUM") as trps,
            tc.tile_pool(name="rps", bufs=2, space="PSUM") as rps,
            tc.tile_pool(name="zcps", bufs=4, space="PSUM") as zcps,
        ):
            idt = cpool.tile([128, 128], BF16, tag="idt")
            make_identity(nc, idt)
            ones = cpool.tile([1, 512], BF16, tag="ones")
            nc.vector.memset(ones, 1.0)

            def emit_weights(g):
                tiles = []
                for j in range(10):
                    kk = 128 if j < 9 else 65
                    t = wpool.tile([128, H], BF16, tag=f"w{g}{j}", name=f"w{g}{j}")
                    nc.sync.dma_start(
                        out=t[:kk, :], in_=W[g][128 * j : 128 * j + kk, :]
                    )
                    tiles.append((t, kk))
                return tiles

            def emit_loads(bidx, eng=None):
                # First blocks ride the gpsimd queue (parallel with the
                # weight stream on sync); later blocks go on sync BEHIND the
                # weights so they can't steal bandwidth from them. Full-row
                # transfers only — 128-col chunks would drop the DMA to
                # ~50GB/s (256B partition lines).
                eng = eng or nc.sync
                t0, btb = blk[bidx]
                nsub = (btb + 127) // 128
                xts = []
                hts = []
                for s in range(nsub):
                    r0 = t0 + s * 128
                    ts_ = min(128, t0 + btb - r0)
                    ht = hpool.tile([128, H], BF16, tag="hnat", name=f"hn{bidx}_{s}")
                    eng.dma_start(out=ht[:ts_, :], in_=h[r0 : r0 + ts_, :])
                    xt = xpool.tile([128, XW], BF16, tag="xnat", name=f"xn{bidx}_{s}")
                    eng.dma_start(out=xt[:ts_, :], in_=x[r0 : r0 + ts_, :])
                    xts.append((xt, ts_))
                    hts.append((ht, ts_))
                return xts, hts

            def emit_transposes(bidx, xts, hts):
                t0, btb = blk[bidx]
                nsub = (btb + 127) // 128
                a = [
                    apool.tile([128, 512], BF16, tag="actT", name=f"actT{bidx}_{j}")
                    for j in range(10)
                ]
                nc.scalar.copy(out=a[9][64:65, :btb], in_=ones[:, :btb])
                for j in range(10):
                    if j < 8:
                        c0, cw = 128 * j, 128
                        srcs = hts
                    elif j == 8:
                        c0, cw = 0, 128
                        srcs = xts
                    else:
                        c0, cw = 128, 64
                        srcs = xts
                    pt = trps.tile([128, 512], BF16, tag="trps")
                    for s in range(nsub):
                        src, ts_ = srcs[s]
                        nc.tensor.transpose(
                            pt[:cw, s * 128 : s * 128 + ts_],
                            src[:ts_, c0 : c0 + cw],
                            idt[:ts_, :ts_],
                        )
                    nc.vector.tensor_copy(out=a[j][:cw, :btb], in_=pt[:cw, :btb])
                return a

            def emit_r(bidx, a, wt):
                btb = blk[bidx][1]
                rh = [
                    rhpool.tile([128, 512], BF16, tag="rh", name=f"rh{bidx}_{k}")
                    for k in range(8)
                ]
                for k in range(8):
                    pr = rps.tile([128, 512], F32, tag="rps")
                    for j, (wtile, kk) in enumerate(wt):
                        nc.tensor.matmul(
                            pr[:, :btb],
                            lhsT=wtile[:kk, 128 * k : 128 * (k + 1)],
                            rhs=a[j][:kk, :btb],
                            start=(j == 0),
                            stop=(j == 9),
                        )
                    nc.scalar.activation(out=rh[k][:, :btb], in_=pr[:, :btb], func=SIG)
                    nc.vector.tensor_mul(
                        rh[k][:, :btb], rh[k][:, :btb], a[k][:128, :btb]
                    )
                return rh

            def emit_z(bidx, a, wt, nsubs):
                btb = blk[bidx][1]
                nsub = (btb + 127) // 128
                zts = []
                for s in range(nsub):
                    ts_ = nsubs[s]
                    zt = zpool.tile([128, H], BF16, tag="zsb", name=f"z{bidx}_{s}")
                    for hh in range(2):
                        pz = zcps.tile([128, 512], F32, tag="zcps")
                        for j, (wtile, kk) in enumerate(wt):
                            nc.tensor.matmul(
                                pz[:ts_, :],
                                lhsT=a[j][:kk, s * 128 : s * 128 + ts_],
                                rhs=wtile[:kk, 512 * hh : 512 * (hh + 1)],
                                start=(j == 0),
                                stop=(j == 9),
                            )
                        nc.scalar.activation(
                            out=zt[:ts_, 512 * hh : 512 * (hh + 1)],
                            in_=pz[:ts_, :],
                            func=SIG,
                        )
                    zts.append(zt)
                return zts

            def emit_c(bidx, a, rh, zts, hts, wt, split_tail=False):
                t0, btb = blk[bidx]
                nsub = (btb + 127) // 128
                for s in range(nsub):
                    ht, ts_ = hts[s]
                    ct = cbpool.tile([128, H], BF16, tag="csb", name=f"c{bidx}_{s}")
                    r0 = t0 + s * 128
                    halfwise = split_tail and s == nsub - 1
                    for hh in range(2):
                        cs = slice(512 * hh, 512 * (hh + 1))
                        pc = zcps.tile([128, 512], F32, tag="zcps")
                        for j, (wtile, kk) in enumerate(wt):
                            lhs_src = rh[j] if j < 8 else a[j]
                            nc.tensor.matmul(
                                pc[:ts_, :],
                                lhsT=lhs_src[:kk, s * 128 : s * 128 + ts_],
                                rhs=wtile[:kk, cs],
                                start=(j == 0),
                                stop=(j == 9),
                            )
                        nc.scalar.activation(
                            out=ct[:ts_, cs], in_=pc[:ts_, :], func=TANH
                        )
                        if halfwise:
                            # tail: combine+store per half so the last store
                            # overlaps the other half's tanh
                            nc.vector.tensor_sub(
                                ct[:ts_, cs], ct[:ts_, cs], ht[:ts_, cs]
                            )
                            nc.vector.tensor_mul(
                                ct[:ts_, cs], ct[:ts_, cs], zts[s][:ts_, cs]
                            )
                            nc.vector.tensor_add(
                                ct[:ts_, cs], ct[:ts_, cs], ht[:ts_, cs]
                            )
                            nc.gpsimd.dma_start(
                                out=out[r0 : r0 + ts_, cs], in_=ct[:ts_, cs]
                            )
                    if not halfwise:
                        # h_new = h + z*(c - h), in place in ct (all bf16)
                        nc.vector.tensor_sub(ct[:ts_, :], ct[:ts_, :], ht[:ts_, :])
                        nc.vector.tensor_mul(ct[:ts_, :], ct[:ts_, :], zts[s][:ts_, :])
                        nc.vector.tensor_add(ct[:ts_, :], ct[:ts_, :], ht[:ts_, :])
                        nc.gpsimd.dma_start(out=out[r0 : r0 + ts_, :], in_=ct[:ts_, :])

            def subs(bidx):
                t0, btb = blk[bidx]
                nsub = (btb + 127) // 128
                return [min(128, btb - s * 128) for s in range(nsub)]

            # ---- startup: activations stream on the gpsimd queue while
            # weights stream on the sync queue; transposes (which need no
            # weights) are interleaved between the first gate GEMMs so the
            # PE has filler work while each gate's weights land; c lags one
            # block throughout.
            state = {}
            state[0] = emit_loads(0, eng=nc.gpsimd)
            state[1] = emit_loads(1, eng=nc.gpsimd)
            wt = {g: emit_weights(g) for g in "zrc"}
            state[2] = emit_loads(2)
            state[3] = emit_loads(3)
            acts = {}
            acts[0] = emit_transposes(0, *state[0])
            acts[1] = emit_transposes(1, *state[1])

            zs = {}
            rhs = {}
            zs[0] = emit_z(0, acts[0], wt["z"], subs(0))
            acts[2] = emit_transposes(2, *state[2])
            rhs[0] = emit_r(0, acts[0], wt["r"])
            acts[3] = emit_transposes(3, *state[3])
            zs[1] = emit_z(1, acts[1], wt["z"], subs(1))
            rhs[1] = emit_r(1, acts[1], wt["r"])
            emit_c(0, acts[0], rhs[0], zs[0], state[0][1], wt["c"])

            for i in range(2, nb):
                if i + 2 < nb:
                    state[i + 2] = emit_loads(i + 2)
                zs[i] = emit_z(i, acts[i], wt["z"], subs(i))
                rhs[i] = emit_r(i, acts[i], wt["r"])
                emit_c(
                    i - 1, acts[i - 1], rhs[i - 1], zs[i - 1], state[i - 1][1],
                    wt["c"],
                )
                if i + 2 < nb:
                    acts[i + 2] = emit_transposes(i + 2, *state[i + 2])
            emit_c(
                nb - 1, acts[nb - 1], rhs[nb - 1], zs[nb - 1], state[nb - 1][1],
                wt["c"], split_tail=True,
            )

    nc.finalize()
    return nc


_NC_CACHE = {}


def _get_nc():
    if "nc" not in _NC_CACHE:
        _NC_CACHE["nc"] = build()
    return _NC_CACHE["nc"]


def _pack_weights(Wg, bg):
    """Rows reordered [h-part, x-part, bias] and cast to bf16."""
    Wg = np.asarray(Wg, dtype=np.float32)
    bg = np.asarray(bg, dtype=np.float32)
    Wp = np.empty((CROWS, H), dtype=np.float32)
    Wp[0:H] = Wg[XW:]
    Wp[H : H + XW] = Wg[:XW]
    Wp[H + XW] = bg
    return np.ascontiguousarray(Wp.astype(ml_dtypes.bfloat16))


def _make_in_maps(diffused_x, h_prev, Wz, bz, Wr, br, Wc, bc):
    x = np.ascontiguousarray(
        np.asarray(diffused_x, dtype=np.float32)
        .reshape(B * N, XW)
        .astype(ml_dtypes.bfloat16)
    )
    hp = np.ascontiguousarray(
        np.asarray(h_prev, dtype=np.float32)
        .reshape(B * N, H)
        .astype(ml_dtypes.bfloat16)
    )
    shared = {
        "Wz": _pack_weights(Wz, bz),
        "Wr": _pack_weights(Wr, br),
        "Wc": _pack_weights(Wc, bc),
    }
    in_maps = []
    for c in range(NCORES):
        sl = slice(c * TPC, (c + 1) * TPC)
        m = {"x": x[sl], "h": hp[sl]}
        m.update(shared)
        in_maps.append(m)
    return in_maps


def kernel(diffused_x, h_prev, Wz, bz, Wr, br, Wc, bc):
    nc = _get_nc()
    in_maps = _make_in_maps(diffused_x, h_prev, Wz, bz, Wr, br, Wc, bc)
    res = run_bass_kernel_spmd(nc, in_maps, list(range(NCORES)))
    outs = [res.results[c]["out"] for c in range(NCORES)]
    return np.concatenate(outs, axis=0).astype(np.float32).reshape(B, N, H)


def kernel_traced(diffused_x, h_prev, Wz, bz, Wr, br, Wc, bc):
    """Like kernel() but with NTFF profiling; returns (out, BassKernelResults)."""
    nc = _get_nc()
    in_maps = _make_in_maps(diffused_x, h_prev, Wz, bz, Wr, br, Wc, bc)
    res = run_bass_kernel_spmd(nc, in_maps, list(range(NCORES)), trace=True)
    outs = [res.results[c]["out"] for c in range(NCORES)]
    return (
        np.concatenate(outs, axis=0).astype(np.float32).reshape(B, N, H),
        res,
    )
